# revision 1
# baseline (speedup 1.0000x reference)
"""Trainium2 Bass kernel for CrossAttention (sparse_attention variant).

Reference computation (shapes hardcoded):
  x [2, 1024, 1024], context [2, 4, 1024, 1024], doc_similarities [2, 4]
  q = x @ Wq, kv = ctx @ Wkv (k|v), dots = q k^T / sqrt(d) + doc_bias,
  attn = softmax(dots over all 4096 doc tokens), out = (attn @ v) @ Wout + bout

Sharding: 8 cores = 2 batches x 4 head-pairs.  Core c: batch c//4, heads
{2*(c%4), 2*(c%4)+1}.  Each core computes a [1024, 1024] partial of the
output projection (its heads' rows of Wout); host sums 4 partials per batch.

On-core layout strategy (all matmuls f32r, full PE rate at free dim 512):
  qT [hd, n], kT [hd, j], vT [hd, j] from projections directly (contraction
  over d with weight tiles stationary, xT/ctxT moving).  S^T tiles [j, i]
  from QK with kT slices stationary.  Softmax along the PSUM partition axis
  (j): exp via ScalarE with fused scale + per-partition doc bias (no max
  subtraction needed; logits are O(5)); denominator via ones-vector matmul;
  V-natural tiles produced on the fly by PE transpose of vT slices
  (software-pipelined one j ahead); EV with V tiles stationary producing
  unnormalized Y^T [hd, i]; normalize by PE-broadcast reciprocal; partial
  output projection with normalized Y^T slices stationary.
"""

import numpy as np
from contextlib import ExitStack

import concourse.bass as bass
import concourse.mybir as mybir
import concourse.tile as tile
from concourse import bacc
from concourse import bass_utils
from concourse.masks import make_identity

# Problem constants
B, N, M, CN, D = 2, 1024, 4, 1024, 1024
H = 8          # total heads
HPC = 2        # heads per core
NCORES = 8
HD = D // H    # 128
J = M * CN     # 4096
KT = D // 128  # 8 contraction k-tiles
IC = N // 512  # 2 i-chunks of queries
JC = J // 512  # 8 j-chunks (projection granularity)
JT = J // 128  # 32 j-tiles (attention granularity)
SCALE = float(D ** -0.5)

FR = mybir.dt.float32r
F32 = mybir.dt.float32

_NC_CACHE = {}
LAST_RESULT = None


def _build_module(reps=1):
    nc = bacc.Bacc(
        "TRN2",
        target_bir_lowering=False,
        debug=False,
        num_devices=NCORES,
    )

    xT = nc.dram_tensor("xT", [D, N], FR, kind="ExternalInput").ap()
    ctxT = nc.dram_tensor("ctxT", [D, J], FR, kind="ExternalInput").ap()
    wq = nc.dram_tensor("wq", [128, KT * HPC * HD], FR, kind="ExternalInput").ap()
    wk = nc.dram_tensor("wk", [128, KT * HPC * HD], FR, kind="ExternalInput").ap()
    wv = nc.dram_tensor("wv", [128, KT * HPC * HD], FR, kind="ExternalInput").ap()
    wout = nc.dram_tensor("wout", [128, HPC * D], FR, kind="ExternalInput").ap()
    docb = nc.dram_tensor("docb", [128, JT], F32, kind="ExternalInput").ap()
    outp = nc.dram_tensor("outp", [N, D], F32, kind="ExternalOutput").ap()

    EXP = mybir.ActivationFunctionType.Exp

    with tile.TileContext(nc) as tc:
        with ExitStack() as ctx:
          wpool = ctx.enter_context(tc.tile_pool(name="wpool", bufs=1))
          big = ctx.enter_context(tc.tile_pool(name="big", bufs=1))
          stream = ctx.enter_context(tc.tile_pool(name="stream", bufs=4))
          epool = ctx.enter_context(tc.tile_pool(name="epool", bufs=4))
          vnpool = ctx.enter_context(tc.tile_pool(name="vnpool", bufs=4))
          spool = ctx.enter_context(tc.tile_pool(name="spool", bufs=2))
          pp = ctx.enter_context(tc.tile_pool(name="pp", bufs=8, space="PSUM"))
          for _rep in range(reps):

              docb_sb = wpool.tile([128, JT], F32, name="docb_sb")
              # memset/affine_select emit invalid ISA for float32r directly;
              # build the constants in f32 and convert via tensor_copy.
              ones_col_f = wpool.tile([128, 1], F32, name="ones_col_f")
              nc.vector.memset(ones_col_f[:, :], 1.0)
              ones_col = wpool.tile([128, 1], FR, name="ones_col")
              nc.vector.tensor_copy(ones_col[:, :], ones_col_f[:, :])
              ones_row_f = wpool.tile([1, 128], F32, name="ones_row_f")
              nc.vector.memset(ones_row_f[:, :], 1.0)
              ones_row = wpool.tile([1, 128], FR, name="ones_row")
              nc.vector.tensor_copy(ones_row[:, :], ones_row_f[:, :])
              ident_f = wpool.tile([128, 128], F32, name="ident_f")
              make_identity(nc, ident_f[:, :])
              ident = wpool.tile([128, 128], FR, name="ident")
              nc.vector.tensor_copy(ident[:, :], ident_f[:, :])

              # weight tiles (DMAs interleaved at first use below)
              wq_sb = wpool.tile([128, KT, HPC * HD], FR, name="wq_sb")
              wk_sb = wpool.tile([128, KT, HPC * HD], FR, name="wk_sb")
              wv_sb = wpool.tile([128, KT, HPC * HD], FR, name="wv_sb")
              wout_sb = wpool.tile([128, HPC, D], FR, name="wout_sb")

              # persistent per-head activations
              qT_sb = big.tile([128, HPC, N], FR, name="qT_sb")    # q^T  [hd, h, i]
              kT_sb = big.tile([128, HPC, J], FR, name="kT_sb")    # k^T  [hd, h, j]
              vT_sb = big.tile([128, HPC, J], FR, name="vT_sb")    # v^T  [hd, h, j]
              yn_sb = big.tile([128, HPC, N], FR, name="yn_sb")    # Ynorm^T [hd, h, i]

              # ---- interleaved Q/K/V projections ----
              # KV(jc) is the bulk; one 2-ktile slice of the Q projection rides
              # along each jc iteration so its DMA spreads across the phase.
              qp = {}
              for s in range(JC):
                  jc = s
                  q_ic = s // 4
                  q_kts = (2 * (s % 4), 2 * (s % 4) + 1)
                  # KV(jc)
                  kp = [
                      pp.tile([128, 512], F32, name=f"kp{h}_{jc}", tag="pp")
                      for h in range(HPC)
                  ]
                  vp = [
                      pp.tile([128, 512], F32, name=f"vp{h}_{jc}", tag="pp")
                      for h in range(HPC)
                  ]
                  for kt in range(KT):
                      if s == 0 and kt % 2 == 0:
                          c = kt // 2
                          nc.sync.dma_start(
                              out=wk_sb[:, 2 * c:2 * c + 2, :],
                              in_=wk[:, c * 512:(c + 1) * 512],
                          )
                          nc.sync.dma_start(
                              out=wv_sb[:, 2 * c:2 * c + 2, :],
                              in_=wv[:, c * 512:(c + 1) * 512],
                          )
                      ct = stream.tile([128, 512], FR, name="ct", tag="ct", bufs=10)
                      nc.sync.dma_start(
                          out=ct[:, :],
                          in_=ctxT[kt * 128:(kt + 1) * 128, jc * 512:(jc + 1) * 512],
                      )
                      if s < 4 and kt == 1:
                          nc.sync.dma_start(
                              out=wq_sb[:, 2 * s:2 * s + 2, :],
                              in_=wq[:, s * 512:(s + 1) * 512],
                          )
                      if s == 0 and kt == 1:
                          nc.sync.dma_start(out=docb_sb[:, :], in_=docb[:, :])
                      for h in range(HPC):
                          nc.tensor.matmul(
                              kp[h][:, :],
                              lhsT=wk_sb[:, kt, h * HD:(h + 1) * HD],
                              rhs=ct[:, :],
                              start=(kt == 0),
                              stop=(kt == KT - 1),
                          )
                          nc.tensor.matmul(
                              vp[h][:, :],
                              lhsT=wv_sb[:, kt, h * HD:(h + 1) * HD],
                              rhs=ct[:, :],
                              start=(kt == 0),
                              stop=(kt == KT - 1),
                          )
                  # Q slice
                  if s % 4 == 0:
                      qp[q_ic] = [
                          pp.tile([128, 512], F32, name=f"qp{h}_{q_ic}", tag="pp")
                          for h in range(HPC)
                      ]
                  for kt in q_kts:
                      xt = stream.tile([128, 512], FR, name="xt", tag="xt", bufs=6)
                      nc.sync.dma_start(
                          out=xt[:, :],
                          in_=xT[kt * 128:(kt + 1) * 128, q_ic * 512:(q_ic + 1) * 512],
                      )
                      for h in range(HPC):
                          nc.tensor.matmul(
                              qp[q_ic][h][:, :],
                              lhsT=wq_sb[:, kt, h * HD:(h + 1) * HD],
                              rhs=xt[:, :],
                              start=(kt == 0),
                              stop=(kt == KT - 1),
                          )
                  for h in range(HPC):
                      nc.vector.tensor_copy(
                          kT_sb[:, h, jc * 512:(jc + 1) * 512], kp[h][:, :]
                      )
                      if s == JC - 1:
                          # final slice: ScalarE is idle until the first exp;
                          # split the eviction backlog across both engines
                          nc.scalar.copy(
                              vT_sb[:, h, jc * 512:(jc + 1) * 512], vp[h][:, :]
                          )
                      else:
                          nc.vector.tensor_copy(
                              vT_sb[:, h, jc * 512:(jc + 1) * 512], vp[h][:, :]
                          )
                  if s % 4 == 3:
                      for h in range(HPC):
                          nc.vector.tensor_copy(
                              qT_sb[:, h, q_ic * 512:(q_ic + 1) * 512],
                              qp[q_ic][h][:, :],
                          )
                      del qp[q_ic]
                  if jc == 0:
                      # out-projection weights: load during the KV phase
                      nc.sync.dma_start(out=wout_sb[:, :, :], in_=wout[:, :])

              # ---- attention, one head at a time ----
              # Each head: QK -> exp (fused scale+bias) -> EV + denominator,
              # software-pipelined one j-tile ahead.  The normalization
              # epilogue of head h is emitted after head h+1's prologue so PE
              # keeps streaming matmuls across the head boundary.
              pending_epilogue = None
              for h in range(HPC):
                  st_tiles = {}
                  vn_tiles = {}

                  def emit_qk(j, h=h, st_tiles=st_tiles):
                      for ic in range(IC):
                          st = pp.tile([128, 512], F32, name=f"st{h}", tag="pp")
                          nc.tensor.matmul(
                              st[:, :],
                              lhsT=kT_sb[:, h, j * 128:(j + 1) * 128],
                              rhs=qT_sb[:, h, ic * 512:(ic + 1) * 512],
                              start=True,
                              stop=True,
                          )
                          st_tiles[(j, ic)] = st

                  def emit_vtrans(j, h=h, vn_tiles=vn_tiles):
                      # V natural tile [j, hd] <- PE transpose of vT slice
                      tp = pp.tile([128, 128], FR, name=f"tp{h}", tag="pp")
                      nc.tensor.transpose(
                          tp[:, :], vT_sb[:, h, j * 128:(j + 1) * 128], ident[:, :]
                      )
                      vn = vnpool.tile([128, 128], FR, name=f"vn{h}", tag="vn")
                      nc.vector.tensor_copy(vn[:, :], tp[:, :])
                      vn_tiles[j] = vn

                  emit_qk(0)
                  emit_vtrans(0)
                  if pending_epilogue is not None:
                      pending_epilogue()
                      pending_epilogue = None
                  y = [
                      pp.tile([128, 512], F32, name=f"y{h}_{ic}", tag="pp")
                      for ic in range(IC)
                  ]
                  dn = [
                      pp.tile([1, 512], F32, name=f"dn{h}_{ic}", tag="pp")
                      for ic in range(IC)
                  ]
                  for j in range(JT):
                      if j + 1 < JT:
                          emit_qk(j + 1)
                          emit_vtrans(j + 1)
                      et = epool.tile([128, N], FR, name=f"et{h}", tag="et")
                      for ic in range(IC):
                          st = st_tiles.pop((j, ic))
                          nc.scalar.activation(
                              et[:, ic * 512:(ic + 1) * 512],
                              st[:, :],
                              EXP,
                              bias=docb_sb[:, j:j + 1],
                              scale=SCALE,
                          )
                      vn = vn_tiles.pop(j)
                      for ic in range(IC):
                          nc.tensor.matmul(
                              y[ic][:, :],
                              lhsT=vn[:, :],
                              rhs=et[:, ic * 512:(ic + 1) * 512],
                              start=(j == 0),
                              stop=(j == JT - 1),
                          )
                          nc.tensor.matmul(
                              dn[ic][:, :],
                              lhsT=ones_col[:, :],
                              rhs=et[:, ic * 512:(ic + 1) * 512],
                              start=(j == 0),
                              stop=(j == JT - 1),
                          )

                  def epilogue(h=h, y=y, dn=dn):
                      # normalize: yn^T = y^T * broadcast(1/denominator)
                      recip = spool.tile([1, N], FR, name=f"recip{h}", tag="recip")
                      for ic in range(IC):
                          with nc.allow_low_precision(
                              reason="float32r output is 32-bit, same as float32"
                          ):
                              nc.vector.reciprocal(
                                  recip[:, ic * 512:(ic + 1) * 512], dn[ic][:, :]
                              )
                      rs = spool.tile([128, N], FR, name=f"rs{h}", tag="rs")
                      for ic in range(IC):
                          bc = pp.tile([128, 512], F32, name=f"bc{h}", tag="pp")
                          nc.tensor.matmul(
                              bc[:, :],
                              lhsT=ones_row[:, :],
                              rhs=recip[:, ic * 512:(ic + 1) * 512],
                              start=True,
                              stop=True,
                          )
                          nc.scalar.copy(rs[:, ic * 512:(ic + 1) * 512], bc[:, :])
                          nc.vector.tensor_mul(
                              yn_sb[:, h, ic * 512:(ic + 1) * 512],
                              y[ic][:, :],
                              rs[:, ic * 512:(ic + 1) * 512],
                          )

                  pending_epilogue = epilogue
              pending_epilogue()

              # ---- partial output projection ----
              for it in range(N // 128):
                  for oc in range(D // 512):
                      op = pp.tile([128, 512], F32, name="op", tag="pp")
                      for h in range(HPC):
                          nc.tensor.matmul(
                              op[:, :],
                              lhsT=yn_sb[:, h, it * 128:(it + 1) * 128],
                              rhs=wout_sb[:, h, oc * 512:(oc + 1) * 512],
                              start=(h == 0),
                              stop=(h == HPC - 1),
                          )
                      ot = stream.tile([128, 512], F32, name="ot", tag="ot")
                      nc.vector.tensor_copy(ot[:, :], op[:, :])
                      nc.sync.dma_start(
                          out=outp[it * 128:(it + 1) * 128, oc * 512:(oc + 1) * 512],
                          in_=ot[:, :],
                      )

    nc.compile()
    return nc


def get_nc(reps=1):
    if reps not in _NC_CACHE:
        _NC_CACHE[reps] = _build_module(reps)
    return _NC_CACHE[reps]


def make_in_maps(inputs):
    x = np.asarray(inputs["x"], dtype=np.float32)
    context = np.asarray(inputs["context"], dtype=np.float32)
    doc = np.asarray(inputs["doc_similarities"], dtype=np.float32)
    cmask = np.asarray(inputs["context_mask"])
    Wq = np.asarray(inputs["Wq"], dtype=np.float32)
    Wkv = np.asarray(inputs["Wkv"], dtype=np.float32)
    beta = float(np.asarray(inputs["beta"]))
    Wout = np.asarray(inputs["Wout"], dtype=np.float32)

    per_batch = []
    for b in range(B):
        xT = np.ascontiguousarray(x[b].T)
        ctxT = np.ascontiguousarray(context[b].reshape(J, D).T)
        bias = np.repeat(doc[b], CN) * beta
        bias = np.where(cmask[b].reshape(J), bias, -1e30).astype(np.float32)
        docb = np.ascontiguousarray(bias.reshape(JT, 128).T)  # [128, JT]
        per_batch.append((xT, ctxT, docb))

    in_maps = []
    for c in range(NCORES):
        b = c // 4
        h0 = (c % 4) * HPC
        xT, ctxT, docb = per_batch[b]
        def pack_kxc(w):
            # [D, C] -> [128, KT*C]: tile rows so each partition line is contiguous
            c = w.shape[1]
            return np.ascontiguousarray(
                w.reshape(KT, 128, c).transpose(1, 0, 2).reshape(128, KT * c)
            )

        wout_c = Wout[h0 * HD:(h0 + HPC) * HD, :]
        in_maps.append({
            "xT": xT,
            "ctxT": ctxT,
            "wq": pack_kxc(Wq[:, h0 * HD:(h0 + HPC) * HD]),
            "wk": pack_kxc(Wkv[:, h0 * HD:(h0 + HPC) * HD]),
            "wv": pack_kxc(Wkv[:, D + h0 * HD:D + (h0 + HPC) * HD]),
            "wout": np.ascontiguousarray(
                wout_c.reshape(HPC, 128, D).transpose(1, 0, 2).reshape(128, HPC * D)
            ),
            "docb": docb,
        })
    return in_maps


def kernel(**inputs):
    global LAST_RESULT
    nc = get_nc()
    in_maps = make_in_maps(inputs)
    res = bass_utils.run_bass_kernel_spmd(
        nc, in_maps, core_ids=list(range(NCORES))
    )
    LAST_RESULT = res
    out = np.zeros((B, N, D), dtype=np.float32)
    for c in range(NCORES):
        out[c // 4] += res.results[c]["outp"]
    out += np.asarray(inputs["bout"], dtype=np.float32)
    return out



# revision 7
# speedup vs baseline: 1.2164x; 1.2164x over previous
"""Trainium2 Bass kernel for CrossAttention (sparse_attention variant).

Reference computation (shapes hardcoded):
  x [2, 1024, 1024], context [2, 4, 1024, 1024], doc_similarities [2, 4]
  q = x @ Wq, kv = ctx @ Wkv (k|v), dots = q k^T / sqrt(d) + doc_bias,
  attn = softmax(dots over all 4096 doc tokens), out = (attn @ v) @ Wout + bout

Sharding: 8 cores = 2 batches x 4 head-pairs.  Core c: batch c//4, heads
{2*(c%4), 2*(c%4)+1}.  Each core computes a [1024, 1024] partial of the
output projection (its heads' rows of Wout); host sums 4 partials per batch.

Implementation notes (all matmul inputs bf16: 1 cycle/row at any free size,
rel err ~6e-3 vs the 2e-2 gate; f32 PSUM accumulation throughout):
  - Single software pipeline over 8 j-chunks of 512 doc tokens: chunk c's
    program projects K/V(c), computes QK+exp for chunk c-1 and E@V for
    chunk c-2.  This overlaps the ScalarE exp stream (~78us total) with
    projection matmuls so PE never waits on softmax.
  - K^T [hd, j] from lhsT=Wk tiles; V directly in natural [j, hd] layout
    from lhsT=ctx^T slices (no PE transposes).
  - Softmax denominator: exp tiles are accumulated elementwise into bf16
    accumulators on DVE (ic=0) and GpSimd (ic=1) -- engines that are
    otherwise idle -- then one tiny ones-vector matmul per (head, ic)
    reduces the 128 j-lanes.  This removes the per-j-tile ones-matmuls
    (64k PE cycles) of the naive approach.  Per-lane bf16 rounding errors
    average out across the 128-lane final reduction (~0.1% on dn).
  - PSUM budget (8 banks): proj ring 3, st ring 3, y ring 2.  E@V
    accumulates per chunk into the y ring and is drained to an f32 SBUF
    accumulator by DVE adds.
  - Normalization via reciprocal + PE row-broadcast, then the partial
    output projection (rows of Wout for this core's heads).
"""

import numpy as np
import ml_dtypes
from contextlib import ExitStack

import concourse.bass as bass
import concourse.mybir as mybir
import concourse.tile as tile
from concourse import bacc
from concourse import bass_utils

# Problem constants
B, N, M, CN, D = 2, 1024, 4, 1024, 1024
H = 8          # total heads
HPC = 2        # heads per core
NCORES = 8
HD = D // H    # 128
J = M * CN     # 4096
KT = D // 128  # 8 contraction k-tiles
IC = N // 512  # 2 i-chunks of queries
JC = J // 512  # 8 j-chunks (pipeline granularity)
JT = J // 128  # 32 j-tiles (attention granularity)
SCALE = float(D ** -0.5)

BF = mybir.dt.bfloat16
FR = mybir.dt.float32r
F32 = mybir.dt.float32
NPBF = ml_dtypes.bfloat16

_NC_CACHE = {}
LAST_RESULT = None


def _build_module(reps=1):
    nc = bacc.Bacc(
        "TRN2",
        target_bir_lowering=False,
        debug=False,
        num_devices=NCORES,
    )

    xT = nc.dram_tensor("xT", [D, N], BF, kind="ExternalInput").ap()
    ctxT = nc.dram_tensor("ctxT", [D, J], BF, kind="ExternalInput").ap()
    wq = nc.dram_tensor("wq", [128, KT * HPC * HD], BF, kind="ExternalInput").ap()
    wk = nc.dram_tensor("wk", [128, KT * HPC * HD], BF, kind="ExternalInput").ap()
    wv = nc.dram_tensor("wv", [128, KT * HPC * HD], BF, kind="ExternalInput").ap()
    wout = nc.dram_tensor("wout", [128, HPC * D], BF, kind="ExternalInput").ap()
    docb = nc.dram_tensor("docb", [128, JT], F32, kind="ExternalInput").ap()
    outp = nc.dram_tensor("outp", [N, D], F32, kind="ExternalOutput").ap()

    EXP = mybir.ActivationFunctionType.Exp

    with tile.TileContext(nc) as tc:
        with ExitStack() as ctx:
          wpool = ctx.enter_context(tc.tile_pool(name="wpool", bufs=1))
          big = ctx.enter_context(tc.tile_pool(name="big", bufs=1))
          stream = ctx.enter_context(tc.tile_pool(name="stream", bufs=4))
          epool = ctx.enter_context(tc.tile_pool(name="epool", bufs=4))
          pp = ctx.enter_context(tc.tile_pool(name="pp", bufs=2, space="PSUM"))
          for _rep in range(reps):
              # ---- constants ----
              ones_col = wpool.tile([128, 1], BF, name="ones_col")
              nc.vector.memset(ones_col[:, :], 1.0)
              # fp32r constants built in f32 then copied (memset can't emit FR)
              ones_row_f = wpool.tile([1, 128], F32, name="ones_row_f")
              nc.vector.memset(ones_row_f[:, :], 1.0)
              ones_row = wpool.tile([1, 128], FR, name="ones_row")
              nc.vector.tensor_copy(ones_row[:, :], ones_row_f[:, :])

              docb_sb = wpool.tile([128, JT], F32, name="docb_sb")

              # ---- weights / activations in SBUF ----
              wq_sb = wpool.tile([128, KT, HPC * HD], BF, name="wq_sb")
              wk_sb = wpool.tile([128, KT, HPC * HD], BF, name="wk_sb")
              wv_sb = wpool.tile([128, KT, HPC * HD], BF, name="wv_sb")
              wout_sb = wpool.tile([128, HPC, D], BF, name="wout_sb")
              xt_sb = wpool.tile([128, KT, N], BF, name="xt_sb")

              qT_sb = big.tile([128, HPC, N], BF, name="qT_sb")     # q^T [hd, h, i]
              kT_sb = big.tile([128, HPC, J], BF, name="kT_sb")     # k^T [hd, h, j]
              vn_sb = big.tile([128, JT * HPC * HD], BF, name="vn_sb")  # v [j, (h hd)]
              yn_sb = big.tile([128, HPC, N], BF, name="yn_sb")     # Ynorm^T [hd, h, i]
              # y accumulator, ping-pong per (h, ic): [pp][128, h, ic, 512]
              y_acc = [
                  big.tile([128, HPC, IC, 512], F32, name=f"y_acc{p}")
                  for p in range(2)
              ]
              # dn accumulators, ping-pong per (h, ic)
              dn_acc = [
                  big.tile([128, HPC, IC, 512], BF, name=f"dn_acc{p}")
                  for p in range(2)
              ]
              rs_sb = big.tile([128, HPC, N], F32, name="rs_sb")
              recip = big.tile([1, HPC, N], FR, name="recip")

              # ---- input DMAs needed up front ----
              nc.sync.dma_start(out=wq_sb[:, :, :], in_=wq[:, :])
              for kt in range(KT):
                  nc.sync.dma_start(
                      out=xt_sb[:, kt, :], in_=xT[kt * 128:(kt + 1) * 128, :]
                  )
              nc.sync.dma_start(out=wk_sb[:, :, :], in_=wk[:, :])
              nc.sync.dma_start(out=wv_sb[:, :, :], in_=wv[:, :])
              nc.sync.dma_start(out=docb_sb[:, :], in_=docb[:, :])

              ct_tiles = {}  # (chunk, kt) -> tile

              def dma_ct(c):
                  for kt in range(KT):
                      t = stream.tile([128, 512], BF, name="ct", tag="ct", bufs=24)
                      nc.sync.dma_start(
                          out=t[:, :],
                          in_=ctxT[kt * 128:(kt + 1) * 128, c * 512:(c + 1) * 512],
                      )
                      ct_tiles[(c, kt)] = t

              # ---- Q projection (with ct(0) DMAs interleaved) ----
              dma_ct(0)
              for ic in range(IC):
                  for h in range(HPC):
                      qp = pp.tile([128, 512], F32, name="qp", tag="proj", bufs=3)
                      for kt in range(KT):
                          nc.tensor.matmul(
                              qp[:, :],
                              lhsT=wq_sb[:, kt, h * HD:(h + 1) * HD],
                              rhs=xt_sb[:, kt, ic * 512:(ic + 1) * 512],
                              start=(kt == 0),
                              stop=(kt == KT - 1),
                          )
                      nc.vector.tensor_copy(
                          qT_sb[:, h, ic * 512:(ic + 1) * 512], qp[:, :]
                      )
              dma_ct(1)

              # ---- fused projection + attention pipeline over j-chunks ----
              et_tiles = {}       # (j, h, ic) -> SBUF bf16 exp tile
              dn_count = {}       # (h, ic) -> adds so far (ping-pong index)
              y_count = {}        # (h, ic) -> chunk-partials folded so far

              def proj_sections(c):
                  # K projection: kT[hd, j-chunk] per head
                  kp = [
                      pp.tile([128, 512], F32, name=f"kp{h}", tag="proj", bufs=3)
                      for h in range(HPC)
                  ]
                  for kt in range(KT):
                      for h in range(HPC):
                          nc.tensor.matmul(
                              kp[h][:, :],
                              lhsT=wk_sb[:, kt, h * HD:(h + 1) * HD],
                              rhs=ct_tiles[(c, kt)][:, :],
                              start=(kt == 0),
                              stop=(kt == KT - 1),
                          )
                  for h in range(HPC):
                      nc.vector.tensor_copy(
                          kT_sb[:, h, c * 512:(c + 1) * 512], kp[h][:, :]
                      )
                  if c == 0:
                      nc.sync.dma_start(out=wout_sb[:, :, :], in_=wout[:, :])
                  if c + 2 <= JC - 1:
                      dma_ct(c + 2)
                  # V projection, natural layout: out [j-slice, (h hd)].
                  # Two j-slices (128 rows each) share one PSUM tile.
                  for pair in range(2):
                      vp = pp.tile([128, 512], F32, name="vp", tag="proj", bufs=3)
                      for sl in (2 * pair, 2 * pair + 1):
                          for kt in range(KT):
                              nc.tensor.matmul(
                                  vp[:, (sl % 2) * 256:(sl % 2) * 256 + 256],
                                  lhsT=ct_tiles[(c, kt)][:, sl * 128:(sl + 1) * 128],
                                  rhs=wv_sb[:, kt, :],
                                  start=(kt == 0),
                                  stop=(kt == KT - 1),
                              )
                      jt = c * 4 + 2 * pair
                      nc.vector.tensor_copy(
                          vn_sb[:, jt * 256:(jt + 2) * 256], vp[:, :]
                      )
                  for kt in range(KT):
                      del ct_tiles[(c, kt)]

              def emit_qk_pair(j, h, deferred_dn):
                  # QK (2 mms) + exp (2 ScalarE) for one (j-tile, head).
                  sts = []
                  for ic in range(IC):
                      st = pp.tile([128, 512], F32, name="st", tag="st", bufs=3)
                      nc.tensor.matmul(
                          st[:, :],
                          lhsT=kT_sb[:, h, j * 128:(j + 1) * 128],
                          rhs=qT_sb[:, h, ic * 512:(ic + 1) * 512],
                          start=True,
                          stop=True,
                      )
                      sts.append(st)
                  for ic in range(IC):
                      et = epool.tile([128, 512], BF, name="et",
                                      tag="et", bufs=40)
                      nc.scalar.activation(
                          et[:, :],
                          sts[ic][:, :],
                          EXP,
                          bias=docb_sb[:, j:j + 1],
                          scale=SCALE,
                      )
                      et_tiles[(j, h, ic)] = et
                      if ic == 0:
                          # DVE dn chain: deferred to end of chunk so the
                          # y-accumulator adds aren't queued behind it
                          deferred_dn.append((j, h, ic))
                      else:
                          emit_dn_add(j, h, ic)

              def emit_dn_add(j, h, ic):
                  et = et_tiles[(j, h, ic)]
                  cnt = dn_count.get((h, ic), 0)
                  src = dn_acc[cnt % 2][:, h, ic, :]
                  dst = dn_acc[(cnt + 1) % 2][:, h, ic, :]
                  eng = nc.vector if ic == 0 else nc.gpsimd
                  if cnt == 0:
                      eng.tensor_copy(dst, et[:, :])
                  else:
                      eng.tensor_add(dst, src, et[:, :])
                  dn_count[(h, ic)] = cnt + 1

              def ev_mm(yp, j, h, ic, jo):
                  nc.tensor.matmul(
                      yp[:, :],
                      lhsT=vn_sb[:, j * 256 + h * HD:j * 256 + (h + 1) * HD],
                      rhs=et_tiles[(j, h, ic)][:, :],
                      start=(jo == 0),
                      stop=(jo == 3),
                  )

              def fold_y(yp, h, ic):
                  cnt = y_count.get((h, ic), 0)
                  dst = y_acc[(cnt + 1) % 2][:, h, ic, :]
                  if cnt == 0:
                      nc.vector.tensor_copy(dst, yp[:, :])
                  else:
                      nc.vector.tensor_add(dst, y_acc[cnt % 2][:, h, ic, :], yp[:, :])
                  y_count[(h, ic)] = cnt + 1

              def attention_section(qc, ec, deferred_dn):
                  # Interleaved [qk-pair, ev, ev] stream: QK+exp for chunk qc,
                  # E@V for chunk ec (either may be None at the pipe ends).
                  # qk pairs: 8 of (j-tile, head); ev: 4 groups of 4 j-mms.
                  qk_sched = [
                      (qc * 4 + jt, h) for jt in range(4) for h in range(HPC)
                  ] if qc is not None else []
                  for half in range(2):
                      hh = half  # ev head for this half
                      yps = {}
                      if ec is not None:
                          for ic in range(IC):
                              yps[ic] = pp.tile([128, 512], F32, name="yp",
                                                tag="yp", bufs=2)
                      for jo in range(4):
                          idx = half * 4 + jo
                          if qc is not None:
                              j, h = qk_sched[idx]
                              emit_qk_pair(j, h, deferred_dn)
                          if ec is not None:
                              for ic in range(IC):
                                  ev_mm(yps[ic], ec * 4 + jo, hh, ic, jo)
                      if ec is not None:
                          for ic in range(IC):
                              fold_y(yps[ic], hh, ic)
                          for jo in range(4):
                              for ic in range(IC):
                                  del et_tiles[(ec * 4 + jo, hh, ic)]

              for c in range(JC + 2):
                  deferred_dn = []
                  if c < JC:
                      proj_sections(c)
                  qc = c - 1 if 1 <= c <= JC else None
                  ec = c - 2 if c >= 2 else None
                  attention_section(qc, ec, deferred_dn)
                  for (j, h, ic) in deferred_dn:
                      emit_dn_add(j, h, ic)

              # ---- epilogue: denominators, normalization ----
              for h in range(HPC):
                  for ic in range(IC):
                      cnt = dn_count[(h, ic)]
                      dnp = pp.tile([1, 512], F32, name="dnp", tag="st", bufs=3)
                      nc.tensor.matmul(
                          dnp[:, :],
                          lhsT=ones_col[:, :],
                          rhs=dn_acc[cnt % 2][:, h, ic, :],
                          start=True,
                          stop=True,
                      )
                      with nc.allow_low_precision(
                          reason="float32r output is 32-bit, same as float32"
                      ):
                          nc.vector.reciprocal(
                              recip[:, h, ic * 512:(ic + 1) * 512], dnp[:, :]
                          )
                  for ic in range(IC):
                      bc = pp.tile([128, 512], F32, name="bc", tag="yp", bufs=2)
                      nc.tensor.matmul(
                          bc[:, :],
                          lhsT=ones_row[:, :],
                          rhs=recip[:, h, ic * 512:(ic + 1) * 512],
                          start=True,
                          stop=True,
                      )
                      nc.scalar.copy(rs_sb[:, h, ic * 512:(ic + 1) * 512], bc[:, :])
                      ycnt = y_count[(h, ic)]
                      nc.vector.tensor_mul(
                          yn_sb[:, h, ic * 512:(ic + 1) * 512],
                          y_acc[ycnt % 2][:, h, ic, :],
                          rs_sb[:, h, ic * 512:(ic + 1) * 512],
                      )

              # ---- partial output projection ----
              for it in range(N // 128):
                  for oc in range(D // 512):
                      op = pp.tile([128, 512], F32, name="op", tag="proj", bufs=3)
                      for h in range(HPC):
                          nc.tensor.matmul(
                              op[:, :],
                              lhsT=yn_sb[:, h, it * 128:(it + 1) * 128],
                              rhs=wout_sb[:, h, oc * 512:(oc + 1) * 512],
                              start=(h == 0),
                              stop=(h == HPC - 1),
                          )
                      ot = stream.tile([128, 512], F32, name="ot", tag="ot", bufs=4)
                      nc.vector.tensor_copy(ot[:, :], op[:, :])
                      nc.sync.dma_start(
                          out=outp[it * 128:(it + 1) * 128, oc * 512:(oc + 1) * 512],
                          in_=ot[:, :],
                      )

    nc.compile()
    return nc


def get_nc(reps=1):
    if reps not in _NC_CACHE:
        _NC_CACHE[reps] = _build_module(reps)
    return _NC_CACHE[reps]


def make_in_maps(inputs):
    x = np.asarray(inputs["x"], dtype=np.float32)
    context = np.asarray(inputs["context"], dtype=np.float32)
    doc = np.asarray(inputs["doc_similarities"], dtype=np.float32)
    cmask = np.asarray(inputs["context_mask"])
    Wq = np.asarray(inputs["Wq"], dtype=np.float32)
    Wkv = np.asarray(inputs["Wkv"], dtype=np.float32)
    beta = float(np.asarray(inputs["beta"]))
    Wout = np.asarray(inputs["Wout"], dtype=np.float32)

    per_batch = []
    for b in range(B):
        xT = np.ascontiguousarray(x[b].T).astype(NPBF)
        ctxT = np.ascontiguousarray(context[b].reshape(J, D).T).astype(NPBF)
        bias = np.repeat(doc[b], CN) * beta
        bias = np.where(cmask[b].reshape(J), bias, -1e30).astype(np.float32)
        docb = np.ascontiguousarray(bias.reshape(JT, 128).T)  # [128, JT]
        per_batch.append((xT, ctxT, docb))

    in_maps = []
    for c in range(NCORES):
        b = c // 4
        h0 = (c % 4) * HPC
        xT, ctxT, docb = per_batch[b]

        def pack_kxc(w):
            # [D, C] -> [128, KT*C]: tile rows so each partition line is contiguous
            cc = w.shape[1]
            return np.ascontiguousarray(
                w.reshape(KT, 128, cc).transpose(1, 0, 2).reshape(128, KT * cc)
            ).astype(NPBF)

        wout_c = Wout[h0 * HD:(h0 + HPC) * HD, :]
        in_maps.append({
            "xT": xT,
            "ctxT": ctxT,
            "wq": pack_kxc(Wq[:, h0 * HD:(h0 + HPC) * HD]),
            "wk": pack_kxc(Wkv[:, h0 * HD:(h0 + HPC) * HD]),
            "wv": pack_kxc(Wkv[:, D + h0 * HD:D + (h0 + HPC) * HD]),
            "wout": np.ascontiguousarray(
                wout_c.reshape(HPC, 128, D).transpose(1, 0, 2).reshape(128, HPC * D)
            ).astype(NPBF),
            "docb": docb,
        })
    return in_maps


def kernel(**inputs):
    global LAST_RESULT
    nc = get_nc()
    in_maps = make_in_maps(inputs)
    res = bass_utils.run_bass_kernel_spmd(
        nc, in_maps, core_ids=list(range(NCORES))
    )
    LAST_RESULT = res
    out = np.zeros((B, N, D), dtype=np.float32)
    for c in range(NCORES):
        out[c // 4] += res.results[c]["outp"]
    out += np.asarray(inputs["bout"], dtype=np.float32)
    return out


# revision 11
# speedup vs baseline: 1.2664x; 1.0410x over previous
"""Trainium2 Bass kernel for CrossAttention (sparse_attention variant).

Reference computation (shapes hardcoded):
  x [2, 1024, 1024], context [2, 4, 1024, 1024], doc_similarities [2, 4]
  q = x @ Wq, kv = ctx @ Wkv (k|v), dots = q k^T / sqrt(d) + doc_bias,
  attn = softmax(dots over all 4096 doc tokens), out = (attn @ v) @ Wout + bout

Sharding: 8 cores = 2 batches x 4 head-pairs.  Core c: batch c//4, heads
{2*(c%4), 2*(c%4)+1}.  Each core computes a [1024, 1024] partial of the
output projection (its heads' rows of Wout); host sums 4 partials per batch.

Implementation notes (all matmul inputs bf16: 1 cycle/row at any free size,
rel err ~6e-3 vs the 2e-2 gate; f32 PSUM accumulation throughout):
  - Single software pipeline over 8 j-chunks of 512 doc tokens: chunk c's
    program projects K/V(c), computes QK+exp for chunk c-1 and E@V for
    chunk c-2.  This overlaps the ScalarE exp stream (~78us total) with
    projection matmuls so PE never waits on softmax.
  - K^T [hd, j] from lhsT=Wk tiles; V directly in natural [j, hd] layout
    from lhsT=ctx^T slices (no PE transposes).
  - Softmax denominator: exp tiles are accumulated elementwise into bf16
    accumulators on DVE (ic=0) and GpSimd (ic=1) -- engines that are
    otherwise idle -- then one tiny ones-vector matmul per (head, ic)
    reduces the 128 j-lanes.  This removes the per-j-tile ones-matmuls
    (64k PE cycles) of the naive approach.  Per-lane bf16 rounding errors
    average out across the 128-lane final reduction (~0.1% on dn).
  - PSUM budget (8 banks): proj ring 3, st ring 3, y ring 2.  E@V
    accumulates per chunk into the y ring and is drained to an f32 SBUF
    accumulator by DVE adds.
  - Normalization via reciprocal + PE row-broadcast, then the partial
    output projection (rows of Wout for this core's heads).
"""

import numpy as np
import ml_dtypes
from contextlib import ExitStack

import concourse.bass as bass
import concourse.mybir as mybir
import concourse.tile as tile
from concourse import bacc
from concourse import bass_utils

# Problem constants
B, N, M, CN, D = 2, 1024, 4, 1024, 1024
H = 8          # total heads
HPC = 2        # heads per core
NCORES = 8
HD = D // H    # 128
J = M * CN     # 4096
KT = D // 128  # 8 contraction k-tiles
IC = N // 512  # 2 i-chunks of queries
JC = J // 512  # 8 j-chunks (pipeline granularity)
JT = J // 128  # 32 j-tiles (attention granularity)
SCALE = float(D ** -0.5)

BF = mybir.dt.bfloat16
FR = mybir.dt.float32r
F32 = mybir.dt.float32
NPBF = ml_dtypes.bfloat16

_NC_CACHE = {}
LAST_RESULT = None


def _build_module(reps=1):
    nc = bacc.Bacc(
        "TRN2",
        target_bir_lowering=False,
        debug=False,
        num_devices=NCORES,
    )

    xT = nc.dram_tensor("xT", [D, N], BF, kind="ExternalInput").ap()
    ctxT = nc.dram_tensor("ctxT", [D, J], BF, kind="ExternalInput").ap()
    wq = nc.dram_tensor("wq", [128, KT * HPC * HD], BF, kind="ExternalInput").ap()
    wk = nc.dram_tensor("wk", [128, KT * HPC * HD], BF, kind="ExternalInput").ap()
    wv = nc.dram_tensor("wv", [128, KT * HPC * HD], BF, kind="ExternalInput").ap()
    wout = nc.dram_tensor("wout", [128, HPC * D], BF, kind="ExternalInput").ap()
    docb = nc.dram_tensor("docb", [128, JT], F32, kind="ExternalInput").ap()
    outp = nc.dram_tensor("outp", [N, D], F32, kind="ExternalOutput").ap()

    EXP = mybir.ActivationFunctionType.Exp

    with tile.TileContext(nc) as tc:
        with ExitStack() as ctx:
          wpool = ctx.enter_context(tc.tile_pool(name="wpool", bufs=1))
          big = ctx.enter_context(tc.tile_pool(name="big", bufs=1))
          stream = ctx.enter_context(tc.tile_pool(name="stream", bufs=4))
          epool = ctx.enter_context(tc.tile_pool(name="epool", bufs=4))
          pp = ctx.enter_context(tc.tile_pool(name="pp", bufs=2, space="PSUM"))
          for _rep in range(reps):
              # ---- constants ----
              ones_col = wpool.tile([128, 1], BF, name="ones_col")
              nc.vector.memset(ones_col[:, :], 1.0)
              # fp32r constants built in f32 then copied (memset can't emit FR)
              ones_row_f = wpool.tile([1, 128], F32, name="ones_row_f")
              nc.vector.memset(ones_row_f[:, :], 1.0)
              ones_row = wpool.tile([1, 128], FR, name="ones_row")
              nc.vector.tensor_copy(ones_row[:, :], ones_row_f[:, :])

              docb_sb = wpool.tile([128, JT], F32, name="docb_sb")

              # ---- weights / activations in SBUF ----
              wq_sb = wpool.tile([128, KT, HPC * HD], BF, name="wq_sb")
              wk_sb = wpool.tile([128, KT, HPC * HD], BF, name="wk_sb")
              wv_sb = wpool.tile([128, KT, HPC * HD], BF, name="wv_sb")
              wout_sb = wpool.tile([128, HPC, D], BF, name="wout_sb")
              xt_sb = wpool.tile([128, KT, N], BF, name="xt_sb")

              qT_sb = big.tile([128, HPC, N], BF, name="qT_sb")     # q^T [hd, h, i]
              kT_sb = big.tile([128, HPC, J], BF, name="kT_sb")     # k^T [hd, h, j]
              vn_sb = big.tile([128, JT * HPC * HD], BF, name="vn_sb")  # v [j, (h hd)]
              yn_sb = big.tile([128, HPC, N], BF, name="yn_sb")     # Ynorm^T [hd, h, i]
              # y accumulator, ping-pong per (h, ic): [pp][128, h, ic, 512]
              y_acc = [
                  big.tile([128, HPC, IC, 512], F32, name=f"y_acc{p}")
                  for p in range(2)
              ]
              # dn accumulators, ping-pong per (h, ic)
              dn_acc = [
                  big.tile([128, HPC, IC, 512], BF, name=f"dn_acc{p}")
                  for p in range(2)
              ]
              rs_sb = big.tile([128, HPC, N], F32, name="rs_sb")
              recip = big.tile([1, HPC, N], FR, name="recip")

              ct_tiles = {}  # (chunk, kt) -> tile

              def dma_ct_one(c, kt):
                  t = stream.tile([128, 512], BF, name="ct", tag="ct", bufs=24)
                  nc.sync.dma_start(
                      out=t[:, :],
                      in_=ctxT[kt * 128:(kt + 1) * 128, c * 512:(c + 1) * 512],
                  )
                  ct_tiles[(c, kt)] = t

              def dma_ct(c):
                  for kt in range(KT):
                      dma_ct_one(c, kt)

              # ---- input DMAs needed up front (xt/ct0 interleaved so both
              # the Q projection and chunk 0 start as early as possible) ----
              nc.sync.dma_start(out=wq_sb[:, :, :], in_=wq[:, :])
              nc.sync.dma_start(out=wk_sb[:, :, :], in_=wk[:, :])
              for kt in range(KT):
                  nc.sync.dma_start(
                      out=xt_sb[:, kt, :], in_=xT[kt * 128:(kt + 1) * 128, :]
                  )
                  dma_ct_one(0, kt)
              nc.sync.dma_start(out=wv_sb[:, :, :], in_=wv[:, :])
              nc.sync.dma_start(out=docb_sb[:, :], in_=docb[:, :])

              # ---- Q projection (kt-outer so the first matmul only waits
              # on the first xt DMA; 4 concurrent PSUM groups) ----
              qps = {}
              for ic in range(IC):
                  for h in range(HPC):
                      tag = "proj" if (ic, h) != (1, 1) else "st"
                      qps[(ic, h)] = pp.tile([128, 512], F32, name="qp",
                                             tag=tag, bufs=3)
              for kt in range(KT):
                  for ic in range(IC):
                      for h in range(HPC):
                          nc.tensor.matmul(
                              qps[(ic, h)][:, :],
                              lhsT=wq_sb[:, kt, h * HD:(h + 1) * HD],
                              rhs=xt_sb[:, kt, ic * 512:(ic + 1) * 512],
                              start=(kt == 0),
                              stop=(kt == KT - 1),
                          )
              for ic in range(IC):
                  for h in range(HPC):
                      nc.vector.tensor_copy(
                          qT_sb[:, h, ic * 512:(ic + 1) * 512],
                          qps[(ic, h)][:, :],
                      )
              dma_ct(1)

              # ---- fused projection + attention pipeline over j-chunks ----
              # Chunk c program: [K proj 16 mms] [kT evicts] then 8 blocks of
              # [4 V mms, qk pair (lag 0), ev pair (lag 1)].  QK for chunk c
              # runs right after its own K eviction; E@V consumes chunk c-1's
              # exp tiles, giving ScalarE a full chunk of slack.
              et_tiles = {}       # (j, h, ic) -> SBUF bf16 exp tile
              dn_count = {}       # (h, ic) -> adds so far (ping-pong index)
              y_count = {}        # (h, ic) -> chunk-partials folded so far

              def emit_qk_pair(j, h, dn_defer):
                  # QK (2 mms) + exp (2 ScalarE) for one (j-tile, head).
                  sts = []
                  for ic in range(IC):
                      st = pp.tile([128, 512], F32, name="st", tag="st", bufs=3)
                      nc.tensor.matmul(
                          st[:, :],
                          lhsT=kT_sb[:, h, j * 128:(j + 1) * 128],
                          rhs=qT_sb[:, h, ic * 512:(ic + 1) * 512],
                          start=True,
                          stop=True,
                      )
                      sts.append(st)
                  for ic in range(IC):
                      et = epool.tile([128, 512], BF, name="et",
                                      tag="et", bufs=40)
                      nc.scalar.activation(
                          et[:, :],
                          sts[ic][:, :],
                          EXP,
                          bias=docb_sb[:, j:j + 1],
                          scale=SCALE,
                      )
                      et_tiles[(j, h, ic)] = et
                      if ic == 0:
                          # DVE dn chain: deferred past the y folds so the
                          # fold adds aren't queued behind it on DVE
                          dn_defer.append((j, h, ic))
                      else:
                          emit_dn_add(j, h, ic)  # GpSimd chain

              def emit_dn_add(j, h, ic):
                  et = et_tiles[(j, h, ic)]
                  cnt = dn_count.get((h, ic), 0)
                  src = dn_acc[cnt % 2][:, h, ic, :]
                  dst = dn_acc[(cnt + 1) % 2][:, h, ic, :]
                  eng = nc.vector if ic == 0 else nc.gpsimd
                  if cnt == 0:
                      eng.tensor_copy(dst, et[:, :])
                  else:
                      eng.tensor_add(dst, src, et[:, :])
                  dn_count[(h, ic)] = cnt + 1

              def ev_mm(yp, j, h, ic, jo):
                  nc.tensor.matmul(
                      yp[:, :],
                      lhsT=vn_sb[:, j * 256 + h * HD:j * 256 + (h + 1) * HD],
                      rhs=et_tiles[(j, h, ic)][:, :],
                      start=(jo == 0),
                      stop=(jo == 3),
                  )

              def fold_y(yp, h, ic):
                  cnt = y_count.get((h, ic), 0)
                  dst = y_acc[(cnt + 1) % 2][:, h, ic, :]
                  if cnt == 0:
                      nc.vector.tensor_copy(dst, yp[:, :])
                  else:
                      nc.vector.tensor_add(dst, y_acc[cnt % 2][:, h, ic, :], yp[:, :])
                  y_count[(h, ic)] = cnt + 1

              def ev_half(ec, half, yps):
                  # allocate the two E@V PSUM groups for this half
                  for ic in range(IC):
                      yps[ic] = pp.tile([128, 512], F32, name="yp",
                                        tag="yp", bufs=2)

              def ev_half_close(ec, half, yps, dn_defer):
                  for ic in range(IC):
                      fold_y(yps[ic], half, ic)
                  for (j, h, ic) in dn_defer:
                      emit_dn_add(j, h, ic)
                  dn_defer.clear()
                  for jo in range(4):
                      for ic in range(IC):
                          del et_tiles[(ec * 4 + jo, half, ic)]

              for c in range(JC):
                  ec = c - 1 if c >= 1 else None
                  # K projection: kT[hd, j-chunk] per head
                  kp = [
                      pp.tile([128, 512], F32, name=f"kp{h}", tag="proj", bufs=3)
                      for h in range(HPC)
                  ]
                  for kt in range(KT):
                      for h in range(HPC):
                          nc.tensor.matmul(
                              kp[h][:, :],
                              lhsT=wk_sb[:, kt, h * HD:(h + 1) * HD],
                              rhs=ct_tiles[(c, kt)][:, :],
                              start=(kt == 0),
                              stop=(kt == KT - 1),
                          )
                  for h in range(HPC):
                      nc.vector.tensor_copy(
                          kT_sb[:, h, c * 512:(c + 1) * 512], kp[h][:, :]
                      )
                  if c == 0:
                      nc.sync.dma_start(out=wout_sb[:, :, :], in_=wout[:, :])
                  if c + 2 <= JC - 1:
                      dma_ct(c + 2)
                  # mixed section: V projection + QK(c) + E@V(c-1)
                  dn_defer = []
                  vps = {}
                  yps = {}
                  for blk in range(8):
                      pair = blk // 4
                      if blk % 4 == 0:
                          vps[pair] = pp.tile([128, 512], F32, name="vp",
                                              tag="proj", bufs=3)
                          if ec is not None:
                              ev_half(ec, pair, yps)
                      # 4 V mms, sl-sequential: one j-slice finishes all its
                      # contraction tiles before the next starts.  The two
                      # slices share one PSUM bank, and start_tensor_calc
                      # zeroes the whole 2KB bank region -- interleaving the
                      # two accumulation groups would corrupt the first.
                      sloff = (blk % 4) // 2
                      sl = 2 * pair + sloff
                      for kt in range(4 * (blk % 2), 4 * (blk % 2) + 4):
                          nc.tensor.matmul(
                              vps[pair][:, sloff * 256:sloff * 256 + 256],
                              lhsT=ct_tiles[(c, kt)][:, sl * 128:(sl + 1) * 128],
                              rhs=wv_sb[:, kt, :],
                              start=(kt == 0),
                              stop=(kt == KT - 1),
                          )
                      # QK pair for this chunk (lag 0)
                      emit_qk_pair(c * 4 + blk // 2, blk % 2, dn_defer)
                      # E@V pair for previous chunk (lag 1)
                      if ec is not None:
                          for ic in range(IC):
                              ev_mm(yps[ic], ec * 4 + blk % 4, pair, ic, blk % 4)
                      if blk % 4 == 3:
                          if ec is not None:
                              ev_half_close(ec, pair, yps, dn_defer)
                          else:
                              for item in dn_defer:
                                  emit_dn_add(*item)
                              dn_defer.clear()
                          jt = c * 4 + 2 * pair
                          nc.vector.tensor_copy(
                              vn_sb[:, jt * 256:(jt + 2) * 256], vps[pair][:, :]
                          )
                  for kt in range(KT):
                      del ct_tiles[(c, kt)]

              # ---- tail: E@V for chunk 7 + per-head epilogues overlapped ----
              def epilogue(h):
                  dnps = []
                  for ic in range(IC):
                      cnt = dn_count[(h, ic)]
                      dnp = pp.tile([1, 512], F32, name="dnp", tag="st", bufs=3)
                      nc.tensor.matmul(
                          dnp[:, :],
                          lhsT=ones_col[:, :],
                          rhs=dn_acc[cnt % 2][:, h, ic, :],
                          start=True,
                          stop=True,
                      )
                      dnps.append(dnp)
                  for ic in range(IC):
                      with nc.allow_low_precision(
                          reason="float32r output is 32-bit, same as float32"
                      ):
                          nc.vector.reciprocal(
                              recip[:, h, ic * 512:(ic + 1) * 512], dnps[ic][:, :]
                          )
                  for ic in range(IC):
                      bc = pp.tile([128, 512], F32, name="bc", tag="yp", bufs=2)
                      nc.tensor.matmul(
                          bc[:, :],
                          lhsT=ones_row[:, :],
                          rhs=recip[:, h, ic * 512:(ic + 1) * 512],
                          start=True,
                          stop=True,
                      )
                      nc.scalar.copy(rs_sb[:, h, ic * 512:(ic + 1) * 512], bc[:, :])
                      ycnt = y_count[(h, ic)]
                      nc.vector.tensor_mul(
                          yn_sb[:, h, ic * 512:(ic + 1) * 512],
                          y_acc[ycnt % 2][:, h, ic, :],
                          rs_sb[:, h, ic * 512:(ic + 1) * 512],
                      )

              ec = JC - 1
              dn_defer = []
              for half in range(HPC):
                  yps = {}
                  ev_half(ec, half, yps)
                  for jo in range(4):
                      for ic in range(IC):
                          ev_mm(yps[ic], ec * 4 + jo, half, ic, jo)
                  ev_half_close(ec, half, yps, dn_defer)
                  # epilogue for head `half` overlaps the other half's E@V
                  epilogue(half)

              # ---- partial output projection ----
              # (h-inner-oc order reuses each yn stationary tile twice;
              # PSUM evictions alternate DVE/ScalarE so neither gates PE)
              for it in range(N // 128):
                  ops = []
                  for oc in range(D // 512):
                      ops.append(pp.tile([128, 512], F32, name="op",
                                         tag="proj", bufs=3))
                  for h in range(HPC):
                      for oc in range(D // 512):
                          nc.tensor.matmul(
                              ops[oc][:, :],
                              lhsT=yn_sb[:, h, it * 128:(it + 1) * 128],
                              rhs=wout_sb[:, h, oc * 512:(oc + 1) * 512],
                              start=(h == 0),
                              stop=(h == HPC - 1),
                          )
                  for oc in range(D // 512):
                      ot = stream.tile([128, 512], F32, name="ot", tag="ot", bufs=6)
                      if (it + oc) % 2 == 0:
                          nc.vector.tensor_copy(ot[:, :], ops[oc][:, :])
                      else:
                          nc.scalar.copy(ot[:, :], ops[oc][:, :])
                      nc.sync.dma_start(
                          out=outp[it * 128:(it + 1) * 128, oc * 512:(oc + 1) * 512],
                          in_=ot[:, :],
                      )

    nc.compile()
    return nc


def get_nc(reps=1):
    if reps not in _NC_CACHE:
        _NC_CACHE[reps] = _build_module(reps)
    return _NC_CACHE[reps]


def make_in_maps(inputs):
    x = np.asarray(inputs["x"], dtype=np.float32)
    context = np.asarray(inputs["context"], dtype=np.float32)
    doc = np.asarray(inputs["doc_similarities"], dtype=np.float32)
    cmask = np.asarray(inputs["context_mask"])
    Wq = np.asarray(inputs["Wq"], dtype=np.float32)
    Wkv = np.asarray(inputs["Wkv"], dtype=np.float32)
    beta = float(np.asarray(inputs["beta"]))
    Wout = np.asarray(inputs["Wout"], dtype=np.float32)

    per_batch = []
    for b in range(B):
        xT = np.ascontiguousarray(x[b].T).astype(NPBF)
        ctxT = np.ascontiguousarray(context[b].reshape(J, D).T).astype(NPBF)
        bias = np.repeat(doc[b], CN) * beta
        bias = np.where(cmask[b].reshape(J), bias, -1e30).astype(np.float32)
        docb = np.ascontiguousarray(bias.reshape(JT, 128).T)  # [128, JT]
        per_batch.append((xT, ctxT, docb))

    in_maps = []
    for c in range(NCORES):
        b = c // 4
        h0 = (c % 4) * HPC
        xT, ctxT, docb = per_batch[b]

        def pack_kxc(w):
            # [D, C] -> [128, KT*C]: tile rows so each partition line is contiguous
            cc = w.shape[1]
            return np.ascontiguousarray(
                w.reshape(KT, 128, cc).transpose(1, 0, 2).reshape(128, KT * cc)
            ).astype(NPBF)

        wout_c = Wout[h0 * HD:(h0 + HPC) * HD, :]
        in_maps.append({
            "xT": xT,
            "ctxT": ctxT,
            "wq": pack_kxc(Wq[:, h0 * HD:(h0 + HPC) * HD]),
            "wk": pack_kxc(Wkv[:, h0 * HD:(h0 + HPC) * HD]),
            "wv": pack_kxc(Wkv[:, D + h0 * HD:D + (h0 + HPC) * HD]),
            "wout": np.ascontiguousarray(
                wout_c.reshape(HPC, 128, D).transpose(1, 0, 2).reshape(128, HPC * D)
            ).astype(NPBF),
            "docb": docb,
        })
    return in_maps


def kernel(**inputs):
    global LAST_RESULT
    nc = get_nc()
    in_maps = make_in_maps(inputs)
    res = bass_utils.run_bass_kernel_spmd(
        nc, in_maps, core_ids=list(range(NCORES))
    )
    LAST_RESULT = res
    out = np.zeros((B, N, D), dtype=np.float32)
    for c in range(NCORES):
        out[c // 4] += res.results[c]["outp"]
    out += np.asarray(inputs["bout"], dtype=np.float32)
    return out
